# revision 1
# baseline (speedup 1.0000x reference)
"""Sparse (log-mask) attention with entmax15 — Trainium2 Bass kernel.

Sharding: 8 cores, core c handles head h=c for both batch rows (data-parallel
over batch inside the core, tensor-parallel over heads across cores).  Each
core computes its head's partial c_proj output; host sums the 8 partials and
adds b_proj.

entmax15 row threshold tau solves sum relu(S - tau)^2 = 4 (raw-score units;
equivalent to the reference's sorted-prefix algorithm).  Solved with secant
iteration on g(tau) = sqrt(sum relu(S-tau)^2) - 2, which is near-linear in
tau, so K_EVALS=6 evaluations reach ~5e-4 output accuracy (7 -> ~1e-5).
Each evaluation is one DVE relu pass + one ACT Square+row-accumulate pass;
the final evaluation's in-place square IS the unnormalized attention, and
1/rowsum is folded into the PSUM->SBUF copy after att @ V.
"""

import numpy as np
import ml_dtypes

B = 2
S = 2048
D = 128
H = 8
QL = 5
NEG = -1e9
NTILE = S // 128  # 16 row tiles
K_EVALS = 6  # secant evaluations of g(tau)
P0, P1 = 2.0, 1.0  # initial points: tau = M - P0, M - P1
GRP = 4  # row tiles per secant batch group

_CACHE = {}


def _build_program(repeat=1):
    import concourse.bass as bass
    import concourse.mybir as mybir
    import concourse.tile as tile
    from concourse import bacc
    from concourse.bass import ts
    from concourse.masks import make_identity

    f32 = mybir.dt.float32
    bf16 = mybir.dt.bfloat16
    AF = mybir.ActivationFunctionType
    OP = mybir.AluOpType

    nc = bacc.Bacc("TRN2", target_bir_lowering=False, debug=False,
                   enable_asserts=False)

    x_d = nc.dram_tensor("x", [B, S, D], f32, kind="ExternalInput").ap()
    wq_d = nc.dram_tensor("wq", [QL, D, D], f32, kind="ExternalInput").ap()
    wk_d = nc.dram_tensor("wk", [QL, D, D], f32, kind="ExternalInput").ap()
    bq_d = nc.dram_tensor("bq", [D, 1], f32, kind="ExternalInput").ap()
    bk_d = nc.dram_tensor("bk", [D, 1], f32, kind="ExternalInput").ap()
    wv_d = nc.dram_tensor("wv", [D, D], f32, kind="ExternalInput").ap()
    bv_d = nc.dram_tensor("bv", [D, 1], f32, kind="ExternalInput").ap()
    wp_d = nc.dram_tensor("wp", [D, D], f32, kind="ExternalInput").ap()
    nm_d = nc.dram_tensor("nmask", [S, S], bf16, kind="ExternalInput").ap()
    po_d = nc.dram_tensor("po", [B, D, S], f32, kind="ExternalOutput").ap()

    with tile.TileContext(nc) as tc:
        for _rep in range(repeat):
            _body(nc, tc, tile, mybir, f32, bf16, AF, OP, ts, make_identity,
                  x_d, wq_d, wk_d, bq_d, bk_d, wv_d, bv_d, wp_d, nm_d, po_d)
    nc.compile()
    return nc


def _body(nc, tc, tile, mybir, f32, bf16, AF, OP, ts, make_identity,
          x_d, wq_d, wk_d, bq_d, bk_d, wv_d, bv_d, wp_d, nm_d, po_d):
    from contextlib import ExitStack

    ctx = ExitStack()
    with ctx:
        cpool = ctx.enter_context(tc.tile_pool(name="consts", bufs=1))
        xpool = ctx.enter_context(tc.tile_pool(name="xn", bufs=3))
        xtp = ctx.enter_context(tc.tile_pool(name="xt", bufs=2))
        qkvp = ctx.enter_context(tc.tile_pool(name="qkv", bufs=2))
        vtp = ctx.enter_context(tc.tile_pool(name="vt", bufs=1))
        spool = ctx.enter_context(tc.tile_pool(name="sc", bufs=GRP + 1))
        ypool = ctx.enter_context(tc.tile_pool(name="yb", bufs=3))
        attp = ctx.enter_context(tc.tile_pool(name="att", bufs=GRP))
        atp = ctx.enter_context(tc.tile_pool(name="attT", bufs=2))
        avp = ctx.enter_context(tc.tile_pool(name="avs", bufs=2))
        pop = ctx.enter_context(tc.tile_pool(name="pot", bufs=1))
        nmp = ctx.enter_context(tc.tile_pool(name="nm", bufs=3))
        stp = ctx.enter_context(tc.tile_pool(name="st", bufs=64))
        ps_big = ctx.enter_context(tc.tile_pool(name="psb", bufs=2, space="PSUM"))
        ps_t = ctx.enter_context(tc.tile_pool(name="pst", bufs=2, space="PSUM"))
        ps_av = ctx.enter_context(tc.tile_pool(name="psav", bufs=1, space="PSUM"))
        ps_pj = ctx.enter_context(tc.tile_pool(name="pspj", bufs=1, space="PSUM"))

        ident = cpool.tile([128, 128], f32, tag="ident")
        make_identity(nc, ident)

        wq_sb = cpool.tile([128, QL * 128], f32, tag="wq")
        wk_sb = cpool.tile([128, QL * 128], f32, tag="wk")
        for t in range(QL):
            nc.sync.dma_start(wq_sb[:, ts(t, 128)], wq_d[t])
            nc.sync.dma_start(wk_sb[:, ts(t, 128)], wk_d[t])
        wv_sb = cpool.tile([128, 128], f32, tag="wv")
        wp_sb = cpool.tile([128, 128], f32, tag="wp")
        nc.sync.dma_start(wv_sb[:], wv_d[:])
        nc.sync.dma_start(wp_sb[:], wp_d[:])
        bq_sb = cpool.tile([128, 1], f32, tag="bq")
        bk_sb = cpool.tile([128, 1], f32, tag="bk")
        bv_sb = cpool.tile([128, 1], f32, tag="bv")
        nc.sync.dma_start(bq_sb[:], bq_d[:])
        nc.sync.dma_start(bk_sb[:], bk_d[:])
        nc.sync.dma_start(bv_sb[:], bv_d[:])

        # xT padded with QL-1 zero columns on the left so every conv tap is a
        # full-range matmul (no partial PSUM accumulation ranges)
        PAD = QL - 1
        xT = []
        for b in range(B):
            xt = xtp.tile([128, S + PAD], f32, tag="xt")
            nc.vector.memset(xt[:, 0:PAD], 0.0)
            for i in range(NTILE):
                xn = xpool.tile([128, 128], f32, tag="xn")
                nc.sync.dma_start(xn[:], x_d[b, ts(i, 128), :])
                pt = ps_t.tile([128, 128], f32, tag="pst")
                nc.tensor.transpose(pt[:], xn[:], ident[:])
                nc.vector.tensor_copy(xt[:, PAD + i * 128: PAD + (i + 1) * 128],
                                      pt[:])
            xT.append(xt)

        # tile order: interleave wide/narrow, grouped in GRP for batched stats
        order = []
        lo, hi = 0, NTILE - 1
        while lo <= hi:
            order.append(hi)
            if lo < hi:
                order.append(lo)
            hi -= 1
            lo += 1
        groups = [order[g:g + GRP] for g in range(0, NTILE, GRP)]

        for b in range(B):
            # causal-conv q/k and v projection, all transposed [feat, s]
            qT = qkvp.tile([128, S], f32, tag="qT")
            kT = qkvp.tile([128, S], f32, tag="kT")
            vT = vtp.tile([128, S], f32, tag="vT")
            for n in range(S // 512):
                for (dst, w_sb, b_sb) in ((qT, wq_sb, bq_sb), (kT, wk_sb, bk_sb)):
                    pq = ps_big.tile([128, 1024], f32, tag="ps")
                    for t in range(QL):
                        sh = QL - 1 - t
                        nc.tensor.matmul(
                            pq[:, 0:512], w_sb[:, ts(t, 128)],
                            xT[b][:, PAD + n * 512 - sh: PAD + n * 512 - sh + 512],
                            start=(t == 0), stop=(t == QL - 1))
                    nc.scalar.activation(dst[:, n * 512:(n + 1) * 512],
                                         pq[:, 0:512], AF.Identity, bias=b_sb[:])
                pv = ps_big.tile([128, 1024], f32, tag="ps")
                nc.tensor.matmul(pv[:, 0:512], wv_sb[:],
                                 xT[b][:, PAD + n * 512: PAD + (n + 1) * 512],
                                 start=True, stop=True)
                nc.scalar.activation(vT[:, n * 512:(n + 1) * 512], pv[:, 0:512],
                                     AF.Identity, bias=bv_sb[:])
            # v natural layout [s, dv] for AV matmul rhs
            v_nat = qkvp.tile([128, S], f32, tag="vnat")
            for j in range(NTILE):
                pt = ps_t.tile([128, 128], f32, tag="pst")
                nc.tensor.transpose(pt[:], vT[:, ts(j, 128)], ident[:])
                nc.vector.tensor_copy(v_nat[:, ts(j, 128)], pt[:])

            poT = pop.tile([128, S], f32, tag="pot")

            for grp in groups:
                G = len(grp)
                # --- scores + rowmax per tile in group ---
                s_list = []
                Mv = stp.tile([128, GRP], f32, tag="st")
                for t, i in enumerate(grp):
                    W = (i + 1) * 128
                    s_sb = spool.tile([128, S], f32, tag="Ssb")
                    nm = nmp.tile([128, S], bf16, tag="nm")
                    nc.sync.dma_start(nm[:, 0:W], nm_d[ts(i, 128), 0:W])
                    nck = (W + 1023) // 1024
                    cmax = stp.tile([128, 2], f32, tag="st")
                    for kc in range(nck):
                        cw = min(1024, W - kc * 1024)
                        pqk = ps_big.tile([128, 1024], f32, tag="ps")
                        for sub in range(0, cw, 512):
                            sw = min(512, cw - sub)
                            nc.tensor.matmul(
                                pqk[:, sub:sub + sw], qT[:, ts(i, 128)],
                                kT[:, kc * 1024 + sub: kc * 1024 + sub + sw],
                                start=True, stop=True)
                        nc.vector.tensor_tensor(
                            s_sb[:, kc * 1024: kc * 1024 + cw], pqk[:, 0:cw],
                            nm[:, kc * 1024: kc * 1024 + cw], OP.add)
                        nc.vector.tensor_reduce(
                            cmax[:, kc:kc + 1], s_sb[:, kc * 1024: kc * 1024 + cw],
                            mybir.AxisListType.X, OP.max)
                    nc.vector.tensor_reduce(Mv[:, t:t + 1], cmax[:, 0:nck],
                                            mybir.AxisListType.X, OP.max)
                    s_list.append(s_sb)

                # --- batched secant on g(tau) = sqrt(sum relu(S-tau)^2)-2 ---
                Mlo = stp.tile([128, GRP], f32, tag="st")
                nc.vector.tensor_scalar_add(Mlo[:, 0:G], Mv[:, 0:G], -2.0)
                ta = stp.tile([128, GRP], f32, tag="st")
                tb = stp.tile([128, GRP], f32, tag="st")
                ga = stp.tile([128, GRP], f32, tag="st")
                gb = stp.tile([128, GRP], f32, tag="st")
                nta = stp.tile([128, GRP], f32, tag="st")
                ntb = stp.tile([128, GRP], f32, tag="st")
                nc.vector.tensor_scalar_add(ta[:, 0:G], Mv[:, 0:G], -P0)
                nc.vector.tensor_scalar_add(tb[:, 0:G], Mv[:, 0:G], -P1)
                nc.vector.tensor_scalar_mul(nta[:, 0:G], ta[:, 0:G], -1.0)
                nc.vector.tensor_scalar_mul(ntb[:, 0:G], tb[:, 0:G], -1.0)

                att_list = [None] * G
                rsum = [None]

                def eval_g(tau_ap, ntau_ap, g_out, last):
                    r2 = stp.tile([128, GRP], f32, tag="st")
                    for t, i in enumerate(grp):
                        W = (i + 1) * 128
                        y = ypool.tile([128, S], f32, tag="yb")
                        nc.vector.tensor_scalar(
                            out=y[:, 0:W], in0=s_list[t][:, 0:W],
                            scalar1=tau_ap[:, t:t + 1], scalar2=ntau_ap[:, t:t + 1],
                            op0=OP.max, op1=OP.add)
                        if last:
                            att = attp.tile([128, S], f32, tag="att")
                            nc.scalar.activation(att[:, 0:W], y[:, 0:W],
                                                 AF.Square,
                                                 accum_out=r2[:, t:t + 1])
                            att_list[t] = att
                        else:
                            nc.scalar.activation(y[:, 0:W], y[:, 0:W], AF.Square,
                                                 accum_out=r2[:, t:t + 1])
                    if last:
                        rsum[0] = r2
                        return
                    nc.scalar.activation(g_out[:, 0:G], r2[:, 0:G], AF.Sqrt)
                    nc.vector.tensor_scalar_add(g_out[:, 0:G], g_out[:, 0:G],
                                                -2.0)

                eval_g(ta, nta, ga, last=False)
                eval_g(tb, ntb, gb, last=False)
                for it in range(K_EVALS - 2):
                    last = (it == K_EVALS - 3)
                    dt = stp.tile([128, GRP], f32, tag="st")
                    dg = stp.tile([128, GRP], f32, tag="st")
                    tn = stp.tile([128, GRP], f32, tag="st")
                    ntn = stp.tile([128, GRP], f32, tag="st")
                    nc.vector.tensor_tensor(dt[:, 0:G], tb[:, 0:G], ta[:, 0:G],
                                            OP.subtract)
                    nc.vector.tensor_tensor(dg[:, 0:G], gb[:, 0:G], ga[:, 0:G],
                                            OP.subtract)
                    # keep dg away from exact 0 (converged rows): recip stays
                    # finite, and the tau clip bounds any junk step
                    nc.vector.tensor_scalar_add(dg[:, 0:G], dg[:, 0:G], -1e-12)
                    nc.vector.reciprocal(dg[:, 0:G], dg[:, 0:G])
                    nc.vector.tensor_tensor(dt[:, 0:G], dt[:, 0:G], dg[:, 0:G],
                                            OP.mult)
                    # slope dt/dg is negative; clamp kills 0/0 junk
                    nc.vector.tensor_scalar(out=dt[:, 0:G], in0=dt[:, 0:G],
                                            scalar1=-1e-9, scalar2=None,
                                            op0=OP.min)
                    nc.vector.tensor_tensor(tn[:, 0:G], gb[:, 0:G], dt[:, 0:G],
                                            OP.mult)
                    nc.vector.tensor_tensor(tn[:, 0:G], tb[:, 0:G], tn[:, 0:G],
                                            OP.subtract)
                    nc.vector.tensor_tensor(tn[:, 0:G], tn[:, 0:G], Mlo[:, 0:G],
                                            OP.max)
                    nc.vector.tensor_tensor(tn[:, 0:G], tn[:, 0:G], Mv[:, 0:G],
                                            OP.min)
                    nc.vector.tensor_scalar_mul(ntn[:, 0:G], tn[:, 0:G], -1.0)
                    ta, ga, nta = tb, gb, ntb
                    tb, ntb = tn, ntn
                    gb = stp.tile([128, GRP], f32, tag="st")
                    eval_g(tb, ntb, gb, last=last)

                # --- normalize + att @ V + proj per tile ---
                for t, i in enumerate(grp):
                    W = (i + 1) * 128
                    att = att_list[t]
                    invr = stp.tile([128, 1], f32, tag="st")
                    nc.vector.tensor_scalar_max(invr[:], rsum[0][:, t:t + 1],
                                                1e-30)
                    nc.vector.reciprocal(invr[:], invr[:])
                    pav = ps_av.tile([128, 128], f32, tag="av")
                    nchunk = i + 1
                    for j0 in range(0, nchunk, 4):
                        m = min(4, nchunk - j0)
                        ptw = ps_t.tile([128, 512], f32, tag="pst")
                        for jj in range(m):
                            nc.tensor.transpose(
                                ptw[:, ts(jj, 128)],
                                att[:, ts(j0 + jj, 128)], ident[:])
                        at_sb = atp.tile([128, 512], f32, tag="attT")
                        nc.vector.tensor_copy(at_sb[:, 0:m * 128],
                                              ptw[:, 0:m * 128])
                        for jj in range(m):
                            j = j0 + jj
                            nc.tensor.matmul(pav[:], at_sb[:, ts(jj, 128)],
                                             v_nat[:, ts(j, 128)],
                                             start=(j == 0),
                                             stop=(j == nchunk - 1))
                    av_sb = avp.tile([128, 128], f32, tag="avs")
                    nc.scalar.activation(av_sb[:], pav[:], AF.Identity,
                                         scale=invr[:])
                    pavt = ps_t.tile([128, 512], f32, tag="pst")
                    nc.tensor.transpose(pavt[:, 0:128], av_sb[:], ident[:])
                    avT = avp.tile([128, 128], f32, tag="avT")
                    nc.vector.tensor_copy(avT[:], pavt[:, 0:128])
                    ppj = ps_pj.tile([128, 128], f32, tag="pj")
                    nc.tensor.matmul(ppj[:], wp_sb[:], avT[:], start=True,
                                     stop=True)
                    nc.vector.tensor_copy(poT[:, ts(i, 128)], ppj[:])
                    nc.sync.dma_start(po_d[b][:, ts(i, 128)],
                                      poT[:, ts(i, 128)])


def _get_program():
    if "nc" not in _CACHE:
        _CACHE["nc"] = _build_program()
    return _CACHE["nc"]


def _make_in_maps(x, mask, w_qk, b_qk, w_v, b_v, w_proj):
    x = np.asarray(x, np.float32)
    mask2d = np.asarray(mask, np.float32).reshape(S, S)
    w_qk = np.asarray(w_qk, np.float32)
    b_qk = np.asarray(b_qk, np.float32)
    w_v = np.asarray(w_v, np.float32)
    b_v = np.asarray(b_v, np.float32)
    w_proj = np.asarray(w_proj, np.float32)
    scale = np.float32(1.0 / np.sqrt(D))
    nmask = ((1.0 - mask2d) * NEG).astype(ml_dtypes.bfloat16)
    in_maps = []
    for c in range(H):
        qs = slice(c * D, (c + 1) * D)
        ks = slice(H * D + c * D, H * D + (c + 1) * D)
        wq = np.ascontiguousarray(
            np.transpose(w_qk[qs], (2, 1, 0))) * scale      # [QL, d, feat]
        wk = np.ascontiguousarray(np.transpose(w_qk[ks], (2, 1, 0)))
        in_maps.append({
            "x": x,
            "wq": wq.astype(np.float32),
            "wk": wk.astype(np.float32),
            "bq": (b_qk[qs] * scale).reshape(D, 1).astype(np.float32),
            "bk": b_qk[ks].reshape(D, 1).astype(np.float32),
            "wv": np.ascontiguousarray(w_v[:, qs]).astype(np.float32),
            "bv": b_v[qs].reshape(D, 1).astype(np.float32),
            "wp": np.ascontiguousarray(w_proj[qs]).astype(np.float32),
            "nmask": nmask,
        })
    return in_maps


def kernel(x, mask, w_qk, b_qk, w_v, b_v, w_proj, b_proj, **_):
    from concourse import bass_utils

    nc = _get_program()
    in_maps = _make_in_maps(x, mask, w_qk, b_qk, w_v, b_v, w_proj)
    res = bass_utils.run_bass_kernel_spmd(nc, in_maps, core_ids=list(range(H)))
    acc = np.zeros((B, D, S), np.float64)
    for r in res.results:
        acc += r["po"].astype(np.float64)
    out = acc.transpose(0, 2, 1).astype(np.float32) + np.asarray(
        b_proj, np.float32)[None, None, :]
    return out

